# revision 1
# baseline (speedup 1.0000x reference)
"""2-layer GCN (GCNConv -> LeakyReLU -> GCNConv) on 8 Trainium2 NeuronCores.

Strategy: dst-partition the graph across 8 cores (each core owns N/8
destination rows and all edges pointing into them). Every core computes the
full dense h' = (x @ W.T) * dis[row] (replicated, cheap; dis = deg^-1/2 is
folded into the PSUM->SBUF copy), writes it row-major bf16 to local HBM,
bulk-gathers h'[src] for its edges with dma_gather (int16 indices; the node
table is split in two halves so indices fit in int16), and aggregates with
0/1 one-hot matmuls accumulated in PSUM; the dst-side dis[dst] scale and the
bias are applied per 128-row block in the epilogue. The symmetric norm
dis[src]*dis[dst] therefore never appears on the per-edge path. Self-loops
are materialized as explicit edges on the host. Between layers the per-core
activations are PE-transposed and AllGathered so layer 2 consumes them
directly as matmul lhsT.
"""

import math
import os as _os

import numpy as np
import ml_dtypes

from concourse import bacc, bass, mybir
import concourse.tile as tile

BF16 = mybir.dt.bfloat16
F32 = mybir.dt.float32
I16 = mybir.dt.int16

NCORES = 8
D = 128
NEG_SLOPE = 0.01
GCMAX = int(_os.environ.get("GCN_GCMAX", "8"))  # chunks per dma_gather call
_NQUEUES = int(_os.environ.get("GCN_NQUEUES", "4"))
_GBUFS = int(_os.environ.get("GCN_GBUFS", "16"))  # gather tile bufs per region
_PREP = bool(int(_os.environ.get("GCN_PREP", "0")))  # prep/trigger split
_PREPW = int(_os.environ.get("GCN_PREPW", "12"))  # prep lookahead window
HGROUP = 8  # h-compute blocks per DMA group


class Plan:
    pass


def make_plan(n_nodes, edge_index):
    """Host-side graph preprocessing: padding, degrees, self-loop edges,
    per-core dst-partitioned + per-(block,half) chunked edge slots."""
    p = Plan()
    src = edge_index[0].astype(np.int64)
    dst = edge_index[1].astype(np.int64)

    unit = NCORES * 128
    p.N = n_nodes
    p.NPAD = ((n_nodes + unit - 1) // unit) * unit
    p.PCN = p.NPAD // NCORES
    p.B = p.PCN // 128
    p.NB = p.NPAD // 128
    p.HALF = p.NPAD // 2
    assert p.HALF - 1 <= 32767, "node count too large for int16 half-split"

    deg = np.bincount(dst, minlength=p.NPAD).astype(np.float32) + 1.0
    dis = (1.0 / np.sqrt(deg)).astype(np.float32)
    p.dis = dis

    # self-loop edges: with the separable norm, a self edge at (i, i) with a
    # 0/1 one-hot contributes dis[i]*h[i]*dis[i] = the reference's analytic
    # self-loop term
    alln = np.arange(p.NPAD, dtype=np.int64)
    src_a = np.concatenate([src, alln])
    dst_a = np.concatenate([dst, alln])

    core = dst_a // p.PCN
    lb = (dst_a % p.PCN) // 128
    dloc = (dst_a % 128).astype(np.float32)
    halfbit = (src_a >= p.HALF).astype(np.int64)
    seg = (core * p.B + lb) * 2 + halfbit
    nseg = NCORES * p.B * 2

    order = np.lexsort((src_a, seg))
    seg_s = seg[order]
    src_s = src_a[order]
    dloc_s = dloc[order]

    counts = np.bincount(seg_s, minlength=nseg)
    cnt = counts.reshape(NCORES, p.B, 2)
    # per-(block,half) chunk counts, shared across cores (max over cores)
    p.chl = [max(1, int(math.ceil(cnt[:, b, 0].max() / 128))) for b in range(p.B)]
    p.chh = [max(1, int(math.ceil(cnt[:, b, 1].max() / 128))) for b in range(p.B)]
    p.SLch = sum(p.chl)
    p.SHch = sum(p.chh)
    p.NCH = p.SLch + p.SHch
    p.STOT = p.NCH * 128
    p.lofs = np.concatenate([[0], np.cumsum(p.chl)])[:-1]
    p.hofs = p.SLch + np.concatenate([[0], np.cumsum(p.chh)])[:-1]

    segid = np.arange(nseg)
    sblk = (segid // 2) % p.B
    sh = segid % 2
    base = np.where(sh == 0, p.lofs[sblk] * 128, p.hofs[sblk] * 128)

    seg_starts = np.zeros(nseg + 1, np.int64)
    np.cumsum(counts, out=seg_starts[1:])
    rank = np.arange(len(seg_s)) - seg_starts[seg_s]
    slot = base[seg_s] + rank
    corefor = seg_s // (2 * p.B)

    idx_all = np.zeros((NCORES, p.STOT), np.int32)
    # pad slots keep dst_local = -1 so is_equal(iota, -1) zeroes their column
    dl_all = np.full((NCORES, p.STOT), -1.0, np.float32)
    val = np.where(src_s >= p.HALF, src_s - p.HALF, src_s)
    idx_all[corefor, slot] = val
    dl_all[corefor, slot] = dloc_s

    # dma_gather index layout: [128, STOT/16] int16, slot s at [s%16, s//16],
    # replicated across the 8 groups of 16 partitions
    idx16 = idx_all.astype(np.int16).reshape(NCORES, p.STOT // 16, 16)
    idx16 = np.ascontiguousarray(idx16.transpose(0, 2, 1))
    p.idx16 = np.ascontiguousarray(np.tile(idx16, (1, 8, 1)))
    # per-chunk dst_local metadata, [128, NCH] with column = chunk
    p.dl = np.ascontiguousarray(dl_all.reshape(NCORES, p.NCH, 128).transpose(0, 2, 1))

    # per-node dis in device layouts
    p.disn = np.ascontiguousarray(
        dis.reshape(p.NB, 128).T
    )  # [128, NB], node nb*128+q at [q, nb]
    p.diso = np.ascontiguousarray(
        dis.reshape(NCORES, p.B, 128).transpose(0, 2, 1)
    )  # [C, 128, B]

    # gather call plan: (is_h, chunk_off_in_global_chunkspace, nchunks)
    p.calls = []
    for is_h, n_region, off in ((0, p.SLch, 0), (1, p.SHch, p.SLch)):
        nc_calls = max(1, math.ceil(n_region / GCMAX))
        per = math.ceil(n_region / nc_calls)
        c0 = 0
        while c0 < n_region:
            cn = min(per, n_region - c0)
            p.calls.append((is_h, off + c0, cn))
            c0 += cn
    p.chunk_call = np.zeros((p.NCH, 2), np.int64)
    for gi, (_, coff, cn) in enumerate(p.calls):
        for c in range(cn):
            p.chunk_call[coff + c] = (gi, c)

    p.key = (p.NPAD, p.B, tuple(p.chl), tuple(p.chh))
    return p


def make_in_maps(plan, x, W1, b1, W2, b2):
    p = plan
    xpad = np.zeros((p.NPAD, D), np.float32)
    xpad[: p.N] = x
    xT = np.ascontiguousarray(xpad.T).astype(ml_dtypes.bfloat16)

    iota = np.tile(np.arange(128, dtype=np.float32)[None, :], (128, 1))
    ident = np.eye(128, dtype=np.float32)

    common = {
        "xT": xT,
        "w1t": np.ascontiguousarray(W1.T).astype(ml_dtypes.bfloat16),
        "w2t": np.ascontiguousarray(W2.T).astype(ml_dtypes.bfloat16),
        "iota": iota.astype(ml_dtypes.bfloat16),
        "ident": ident.astype(ml_dtypes.bfloat16),
        "bias1": np.ascontiguousarray(
            np.tile(np.asarray(b1, np.float32)[None, :], (128, 1))
        ),
        "bias2": np.ascontiguousarray(
            np.tile(np.asarray(b2, np.float32)[None, :], (128, 1))
        ),
        "disn": p.disn,
    }
    return [
        dict(common, idx=p.idx16[k], dln=p.dl[k], diso=p.diso[k])
        for k in range(NCORES)
    ]


def build_program(plan):
    p = plan

    nc = bacc.Bacc(
        "TRN2",
        target_bir_lowering=False,
        debug=False,
        num_devices=NCORES,
        num_swdge_queues=_NQUEUES,
    )

    xT_d = nc.dram_tensor("xT", [128, p.NPAD], BF16, kind="ExternalInput")
    w1t_d = nc.dram_tensor("w1t", [128, 128], BF16, kind="ExternalInput")
    w2t_d = nc.dram_tensor("w2t", [128, 128], BF16, kind="ExternalInput")
    iota_d = nc.dram_tensor("iota", [128, 128], BF16, kind="ExternalInput")
    ident_d = nc.dram_tensor("ident", [128, 128], BF16, kind="ExternalInput")
    bias1_d = nc.dram_tensor("bias1", [128, 128], F32, kind="ExternalInput")
    bias2_d = nc.dram_tensor("bias2", [128, 128], F32, kind="ExternalInput")
    disn_d = nc.dram_tensor("disn", [128, p.NB], F32, kind="ExternalInput")
    diso_d = nc.dram_tensor("diso", [128, p.B], F32, kind="ExternalInput")
    idx_d = nc.dram_tensor("idx", [128, p.STOT // 16], I16, kind="ExternalInput")
    dln_d = nc.dram_tensor("dln", [128, p.NCH], F32, kind="ExternalInput")
    out_d = nc.dram_tensor("out", [p.PCN, 128], F32, kind="ExternalOutput")

    with tile.TileContext(nc) as tc:
        with (
            tc.tile_pool(name="dram", bufs=1, space="DRAM") as dpool,
            tc.tile_pool(name="const", bufs=1) as cpool,
            tc.tile_pool(name="work", bufs=2) as wpool,
            tc.tile_pool(name="psum", bufs=2, space="PSUM") as pspool,
        ):
            # each h table is split into lo/hi halves as separate tensors so
            # the L-region gathers depend only on the lower half's writes
            # (whole-tensor deps would stall gathers behind the full h phase)
            h1lo_t = dpool.tile([p.HALF, 128], BF16, name="h1lo")
            h1hi_t = dpool.tile([p.HALF, 128], BF16, name="h1hi")
            h2lo_t = dpool.tile([p.HALF, 128], BF16, name="h2lo")
            h2hi_t = dpool.tile([p.HALF, 128], BF16, name="h2hi")
            agin_t = dpool.tile([128, p.PCN], BF16, name="aginbuf")
            # (addr_space="Shared" would be faster for the collective, but
            # neuronxcc's DataLocalityOpt crashes on DMA loads from Shared
            # scratchpad tensors)
            agout_t = dpool.tile([NCORES * 128, p.PCN], BF16, name="agoutbuf")

            def cload(dram, shape, dtype, name):
                t = cpool.tile(shape, dtype, name=name)
                nc.sync.dma_start(out=t[:], in_=dram.ap())
                return t

            w1t_s = cload(w1t_d, [128, 128], BF16, "w1t_s")
            w2t_s = cload(w2t_d, [128, 128], BF16, "w2t_s")
            iota_s = cload(iota_d, [128, 128], BF16, "iota_s")
            ident_s = cload(ident_d, [128, 128], BF16, "ident_s")
            bias1_s = cload(bias1_d, [128, 128], F32, "bias1_s")
            bias2_s = cload(bias2_d, [128, 128], F32, "bias2_s")
            disn_s = cload(disn_d, [128, p.NB], F32, "disn_s")
            diso_s = cload(diso_d, [128, p.B], F32, "diso_s")
            idx_s = cload(idx_d, [128, p.STOT // 16], I16, "idx_s")
            dln_s = cload(dln_d, [128, p.NCH], F32, "dln_s")

            # --- prep/trigger pipelined gathers (both layers unified) ---
            # Descriptor generation (Q7 software, the serial bottleneck) has
            # no dependency on the h tables, so prepare_only preps run far
            # ahead — layer 2's generation overlaps layer 1's compute. The
            # trigger carries the h-table read dependency instead.
            all_calls = [
                (layer, is_h, coff, cn)
                for layer in (1, 2)
                for (is_h, coff, cn) in p.calls
            ]
            qsems = (
                [nc.alloc_semaphore(f"gsem{q}") for q in range(_NQUEUES)]
                if _PREP
                else None
            )
            gstate = {"prep": 0, "trig": 0, "tiles": {}}

            def emit_prep(k):
                layer, is_h, coff, cn = all_calls[k]
                htab_k = None
                gt = wpool.tile(
                    [128, GCMAX, 128],
                    BF16,
                    tag=("gbufH" if is_h else "gbufL"),
                    bufs=_GBUFS,
                    name="gt",
                )
                tab = (
                    (h1hi_t if layer == 1 else h2hi_t)
                    if is_h
                    else (h1lo_t if layer == 1 else h2lo_t)
                )[:, :]
                ni = cn * 128
                soff = coff * 128
                q = k % _NQUEUES
                nc.gpsimd.dma_gather(
                    gt[:, :cn, :],
                    tab,
                    idx_s[:, soff // 16 : (soff + ni) // 16],
                    ni,
                    ni,
                    128,
                    elem_step=128,
                    single_packet=(ni <= 1024),
                    queue_num=q,
                    prepare_only=True,
                    sem=qsems[q],
                )
                gstate["tiles"][k] = gt

            # Tile only wires consumer waits for trigger_dma(count=None),
            # which fires a queue's whole pending set — so a pending set must
            # never mix layers (an L2 prep's h2 dependency on a trigger that
            # L1 consumers wait on would deadlock). Prep emission is capped at
            # the layer boundary until that layer is flushed.
            def emit_preps_until(limit, cap):
                while gstate["prep"] < min(cap, len(all_calls)):
                    if gstate["prep"] >= limit:
                        break
                    emit_prep(gstate["prep"])
                    gstate["prep"] += 1

            def _fire_all_pending():
                for q in range(_NQUEUES):
                    if any(
                        t % _NQUEUES == q
                        for t in range(gstate["trig"], gstate["prep"])
                    ):
                        nc.gpsimd.trigger_dma(count=None, queue_num=q)
                gstate["trig"] = gstate["prep"]

            def ensure_triggered(k, layer_cap):
                if k >= gstate["trig"]:
                    _fire_all_pending()
                    emit_preps_until(layer_cap, gstate["trig"] + _PREPW)

            def flush_layer(layer_cap):
                _fire_all_pending()
                emit_preps_until(layer_cap, len(all_calls))

            if _PREP:
                emit_preps_until(len(p.calls), _PREPW)

            def h_phase(layer):
                """h' = (lhs @ W.T) * dis[row] for all NPAD nodes -> DRAM."""
                wst = w1t_s if layer == 1 else w2t_s
                hlo = h1lo_t if layer == 1 else h2lo_t
                hhi = h1hi_t if layer == 1 else h2hi_t
                nbh = p.NB // 2  # first block of the upper half
                groups = []
                if layer == 1:
                    nb0 = 0
                    while nb0 < p.NB:
                        gs = min(HGROUP, p.NB - nb0)
                        if nb0 < nbh < nb0 + gs:
                            gs = nbh - nb0  # don't straddle the half boundary
                        groups.append((nb0, gs))
                        nb0 += gs
                else:
                    # lhs tiles come from the AllGather output; groups must
                    # not cross rank boundaries
                    for r in range(NCORES):
                        lb0 = 0
                        while lb0 < p.B:
                            gs = min(HGROUP, p.B - lb0)
                            groups.append((r * p.B + lb0, gs))
                            lb0 += gs
                for nb0, gs in groups:
                    lhs_g = wpool.tile(
                        [128, HGROUP * 128], BF16, tag="lhsg", name="lhs_g"
                    )
                    if layer == 1:
                        nc.sync.dma_start(
                            out=lhs_g[:, : gs * 128],
                            in_=xT_d[:, nb0 * 128 : (nb0 + gs) * 128],
                        )
                    else:
                        r = nb0 // p.B
                        lb0 = nb0 - r * p.B
                        nc.sync.dma_start(
                            out=lhs_g[:, : gs * 128],
                            in_=agout_t[
                                r * 128 : (r + 1) * 128,
                                lb0 * 128 : (lb0 + gs) * 128,
                            ],
                        )
                    hst = wpool.tile(
                        [128, HGROUP, 128], BF16, tag="hst", bufs=3, name="hst"
                    )
                    for j in range(gs):
                        nb = nb0 + j
                        ps = pspool.tile(
                            [128, 128], F32, tag="hps", bufs=4, name="hps"
                        )
                        nc.tensor.matmul(
                            out=ps[:],
                            lhsT=lhs_g[:, j * 128 : (j + 1) * 128],
                            rhs=wst[:],
                            start=True,
                            stop=True,
                        )
                        # PSUM -> SBUF copy doubles as the dis[row] scale
                        if j % 2 == 0:
                            nc.scalar.mul(
                                out=hst[:, j, :], in_=ps[:],
                                mul=disn_s[:, nb : nb + 1],
                            )
                        else:
                            nc.vector.tensor_scalar(
                                hst[:, j, :],
                                ps[:],
                                disn_s[:, nb : nb + 1],
                                None,
                                mybir.AluOpType.mult,
                            )
                    if nb0 >= nbh:
                        hw_dst, row0 = hhi, (nb0 - nbh) * 128
                    else:
                        hw_dst, row0 = hlo, nb0 * 128
                    nc.sync.dma_start(
                        out=hw_dst[row0 : row0 + gs * 128, :].rearrange(
                            "(j q) f -> q j f", q=128
                        ),
                        in_=hst[:, :gs, :],
                    )

            def edge_phase(layer):
                bias_s = bias1_s if layer == 1 else bias2_s
                # bulk gathers of h'[src]. L and H regions rotate separate
                # buffer tags (a block consumes an early L call together with
                # a late H call; one shared tag deadlocks the rotation).
                cbase = 0 if layer == 1 else len(p.calls)
                call_tiles = []
                if not _PREP:
                    for gi, (is_h, coff, cn) in enumerate(p.calls):
                        gt = wpool.tile(
                            [128, GCMAX, 128],
                            BF16,
                            tag=("gbufH" if is_h else "gbufL"),
                            bufs=_GBUFS,
                            name="gt",
                        )
                        tab = (
                            (h1hi_t if layer == 1 else h2hi_t)
                            if is_h
                            else (h1lo_t if layer == 1 else h2lo_t)
                        )[:, :]
                        ni = cn * 128
                        soff = coff * 128
                        nc.gpsimd.dma_gather(
                            gt[:, :cn, :],
                            tab,
                            idx_s[:, soff // 16 : (soff + ni) // 16],
                            ni,
                            ni,
                            128,
                            elem_step=128,
                            single_packet=(ni <= 1024),
                            queue_num=gi % _NQUEUES,
                        )
                        call_tiles.append(gt)

                if layer == 1:
                    a1st = wpool.tile(
                        [128, p.B * 128], BF16, tag="a1st", bufs=1, name="a1st"
                    )
                else:
                    outst = wpool.tile(
                        [128, p.B, 128], F32, tag="outst", bufs=1, name="outst"
                    )

                for b in range(p.B):
                    agg = pspool.tile([128, 128], F32, tag="agg", name="agg")
                    chunk_ids = [p.lofs[b] + c for c in range(p.chl[b])] + [
                        p.hofs[b] + c for c in range(p.chh[b])
                    ]
                    nch = len(chunk_ids)
                    if _PREP:
                        ensure_triggered(
                            cbase + max(p.chunk_call[ci][0] for ci in chunk_ids),
                            (layer - 1) * len(p.calls) + len(p.calls),
                        )
                    for k, ci in enumerate(chunk_ids):
                        pt = wpool.tile(
                            [128, 128], BF16, tag="ptile", bufs=4, name="pt"
                        )
                        nc.vector.tensor_scalar(
                            pt[:],
                            iota_s[:],
                            dln_s[:, ci : ci + 1],
                            None,
                            mybir.AluOpType.is_equal,
                        )
                        gi, c = p.chunk_call[ci]
                        gtile = (
                            gstate["tiles"][cbase + gi] if _PREP else call_tiles[gi]
                        )
                        nc.tensor.matmul(
                            out=agg[:],
                            lhsT=pt[:],
                            rhs=gtile[:, c : c + 1, :],
                            start=(k == 0),
                            stop=(k == nch - 1),
                        )
                    # epilogue: dst-side dis scale + bias (+ leaky relu)
                    t1 = wpool.tile([128, 128], F32, tag="ep1", name="t1")
                    nc.vector.tensor_scalar(
                        t1[:],
                        agg[:],
                        diso_s[:, b : b + 1],
                        None,
                        mybir.AluOpType.mult,
                    )
                    t2 = wpool.tile([128, 128], F32, tag="ep2", name="t2")
                    nc.vector.tensor_tensor(
                        out=t2[:], in0=t1[:], in1=bias_s[:], op=mybir.AluOpType.add
                    )
                    if layer == 1:
                        t3 = wpool.tile([128, 128], F32, tag="ep3", name="t3")
                        nc.vector.tensor_scalar(
                            t3[:], t2[:], NEG_SLOPE, None, mybir.AluOpType.mult
                        )
                        a1b = wpool.tile([128, 128], BF16, tag="a1b", name="a1b")
                        nc.vector.tensor_tensor(
                            out=a1b[:], in0=t2[:], in1=t3[:], op=mybir.AluOpType.max
                        )
                        tp = pspool.tile(
                            [128, 128], BF16, space="PSUM", tag="tp", name="tp"
                        )
                        nc.tensor.transpose(
                            out=tp[:], in_=a1b[:], identity=ident_s[:]
                        )
                        if b % 2 == 0:
                            nc.scalar.copy(
                                out=a1st[:, b * 128 : (b + 1) * 128], in_=tp[:]
                            )
                        else:
                            nc.vector.tensor_copy(
                                out=a1st[:, b * 128 : (b + 1) * 128], in_=tp[:]
                            )
                    else:
                        nc.vector.tensor_copy(out=outst[:, b, :], in_=t2[:])

                if _PREP:
                    flush_layer(len(all_calls))
                if layer == 1:
                    return a1st
                nc.sync.dma_start(
                    out=out_d.ap().rearrange("(b q) f -> q b f", q=128),
                    in_=outst[:, :, :],
                )
                return None

            def ag_phase(a1st):
                nc.sync.dma_start(out=agin_t[:, :], in_=a1st[:, :])
                nc.gpsimd.collective_compute(
                    "AllGather",
                    mybir.AluOpType.bypass,
                    replica_groups=[list(range(NCORES))],
                    ins=[agin_t[:, :].opt()],
                    outs=[agout_t[:, :].opt()],
                )

            h_phase(1)
            a1st = edge_phase(1)
            ag_phase(a1st)
            h_phase(2)
            edge_phase(2)

    nc.compile()
    return nc


_CACHE = {}


def _get_program(plan):
    nc = _CACHE.get(plan.key)
    if nc is None:
        nc = build_program(plan)
        _CACHE[plan.key] = nc
    return nc


def kernel(x, edge_index, batch, W1, b1, W2, b2):
    from concourse.bass_utils import run_bass_kernel_spmd

    x = np.asarray(x, np.float32)
    edge_index = np.asarray(edge_index)
    plan = make_plan(x.shape[0], edge_index)
    in_maps = make_in_maps(
        plan,
        x,
        np.asarray(W1, np.float32),
        np.asarray(b1, np.float32),
        np.asarray(W2, np.float32),
        np.asarray(b2, np.float32),
    )
    nc = _get_program(plan)
    res = run_bass_kernel_spmd(nc, in_maps, core_ids=list(range(NCORES)))
    out = np.concatenate([res.results[k]["out"] for k in range(NCORES)], axis=0)
    return np.ascontiguousarray(out[: plan.N]).astype(np.float32)



# revision 7
# speedup vs baseline: 1.3845x; 1.3845x over previous
"""2-layer GCN (GCNConv -> LeakyReLU -> GCNConv) on 8 Trainium2 NeuronCores.

v2: aggregate-then-transform. GCN's aggregation commutes with the linear map
(A_norm @ (X W) == (A_norm @ X) W), so each layer gathers RAW (pre-scaled)
node features and applies W once per 128-dst block afterwards:

  - dst-partition the graph across 8 cores; host ships xd = x*dis (bf16 rows)
    as the layer-1 gather table, so layer-1 gathers start immediately (no
    on-device dense phase before them).
  - per dst block: aggXT[c,d] = sum_slots gathered[slot,c]*onehot[slot,d]
    accumulated in PSUM via one matmul per 128-slot chunk (lhsT = gathered
    tile, rhs = one-hot). One-hot tiles are built in batches of GCMAX chunks
    with a single broadcast is_equal per gather call.
  - self-loops are folded analytically: u = aggXT + (x*dis^2)[:,block] in the
    PSUM->SBUF copy (one tensor_tensor add); no self-loop gather slots.
  - v = W.T-matmul (stationary w per layer); epilogue applies the dst-side
    dis scale, bias, and leaky-relu in feature-major space; layer-1 output
    rows (a1*dis, the layer-2 table values) are produced via one 128x128
    SBUF->SBUF DMA-transpose per block into a row-major staging tile.
  - a1 rows -> AllGather([PCN,128] -> [NPAD,128]); the collective output IS
    the layer-2 gather table (no rebuild).
  - layer-2 output stays feature-major; the host transposes and un-permutes.

Nodes are relabeled on the host (snake assignment over degree-sorted nodes)
so per-(core,block,half) edge counts are balanced: the chunk schedule is
shared across cores, so padding is set by the max count - balancing makes
max ~= mean and cuts ~13% of gather slots vs. naive labeling.
"""

import math
import os as _os

import numpy as np
import ml_dtypes

from concourse import bacc, bass, mybir
import concourse.tile as tile

BF16 = mybir.dt.bfloat16
F32 = mybir.dt.float32
I16 = mybir.dt.int16

NCORES = 8
D = 128
NEG_SLOPE = 0.01
GCMAX = int(_os.environ.get("GCN_GCMAX", "12"))  # chunks per dma_gather call
_NQUEUES = int(_os.environ.get("GCN_NQUEUES", "4"))
_GBUFS = int(_os.environ.get("GCN_GBUFS", "5"))  # gather tile bufs per region
_PTBUFS = int(_os.environ.get("GCN_PTBUFS", "3"))
_SCRATCH = int(_os.environ.get("GCN_SCRATCH", "24576"))


class Plan:
    pass


def make_plan(n_nodes, edge_index):
    p = Plan()
    src = edge_index[0].astype(np.int64)
    dst = edge_index[1].astype(np.int64)

    unit = NCORES * 128
    p.N = n_nodes
    p.NPAD = ((n_nodes + unit - 1) // unit) * unit
    p.PCN = p.NPAD // NCORES
    p.B = p.PCN // 128
    p.NB = p.NPAD // NCORES * NCORES // 128
    p.NB = p.NPAD // 128
    p.HALF = p.NPAD // 2
    assert p.HALF - 1 <= 32767, "node count too large for int16 half-split"

    deg = np.bincount(dst, minlength=p.NPAD).astype(np.float64) + 1.0
    dis = (1.0 / np.sqrt(deg)).astype(np.float32)
    p.dis = dis

    # snake relabeling: sort nodes by in-degree, deal one per bin per round
    # (alternating direction) -> every 128-node block has ~equal total degree
    order = np.argsort(-deg, kind="stable")
    arr = order.reshape(128, p.NB).copy()
    arr[1::2] = arr[1::2, ::-1]
    newid = np.empty(p.NPAD, np.int64)
    newid[arr] = (np.arange(p.NB)[None, :] * 128 + np.arange(128)[:, None])
    node_at = np.empty(p.NPAD, np.int64)
    node_at[newid] = np.arange(p.NPAD)
    p.newid = newid
    p.node_at = node_at

    src_n = newid[src]
    dst_n = newid[dst]

    core = dst_n // p.PCN
    lb = (dst_n % p.PCN) // 128
    dloc = (dst_n % 128).astype(np.float32)
    halfbit = (src_n >= p.HALF).astype(np.int64)
    seg = (core * p.B + lb) * 2 + halfbit
    nseg = NCORES * p.B * 2

    sorder = np.lexsort((src_n, seg))
    seg_s = seg[sorder]
    src_s = src_n[sorder]
    dloc_s = dloc[sorder]

    counts = np.bincount(seg_s, minlength=nseg)
    cnt = counts.reshape(NCORES, p.B, 2)
    p.chl = [max(1, int(math.ceil(cnt[:, b, 0].max() / 128))) for b in range(p.B)]
    p.chh = [max(1, int(math.ceil(cnt[:, b, 1].max() / 128))) for b in range(p.B)]
    p.SLch = sum(p.chl)
    p.SHch = sum(p.chh)
    p.NCH = p.SLch + p.SHch
    p.STOT = p.NCH * 128
    p.lofs = np.concatenate([[0], np.cumsum(p.chl)])[:-1]
    p.hofs = p.SLch + np.concatenate([[0], np.cumsum(p.chh)])[:-1]

    segid = np.arange(nseg)
    sblk = (segid // 2) % p.B
    sh = segid % 2
    base = np.where(sh == 0, p.lofs[sblk] * 128, p.hofs[sblk] * 128)

    seg_starts = np.zeros(nseg + 1, np.int64)
    np.cumsum(counts, out=seg_starts[1:])
    rank = np.arange(len(seg_s)) - seg_starts[seg_s]
    slot = base[seg_s] + rank
    corefor = seg_s // (2 * p.B)

    idx_all = np.zeros((NCORES, p.STOT), np.int32)
    # pad slots: idx 0 (safe row), dst_local -1 so is_equal zeroes the column
    dl_all = np.full((NCORES, p.STOT), -1.0, np.float32)
    val = np.where(src_s >= p.HALF, src_s - p.HALF, src_s)
    idx_all[corefor, slot] = val
    dl_all[corefor, slot] = dloc_s

    # gather call plan: (is_h, chunk_off, nchunks), GCMAX chunks per call
    p.calls = []
    for is_h, n_region, off in ((0, p.SLch, 0), (1, p.SHch, p.SLch)):
        nc_calls = max(1, math.ceil(n_region / GCMAX))
        per = math.ceil(n_region / nc_calls)
        c0 = 0
        while c0 < n_region:
            cn = min(per, n_region - c0)
            p.calls.append((is_h, off + c0, cn))
            c0 += cn
    p.chunk_call = np.zeros((p.NCH, 2), np.int64)
    for gi, (_, coff, cn) in enumerate(p.calls):
        for c in range(cn):
            p.chunk_call[coff + c] = (gi, c)

    # mark pad slots at each call's tail as -1: the Q7 kernel trims trailing
    # negatives, skipping their descriptor generation + DMA entirely. The
    # num_idxs register is a shared program constant, so only the tail run
    # that is padding on EVERY core can be trimmed (mid-stream negatives are
    # NOT safe - uint32 address math - tails only).
    occupied = np.zeros((NCORES, p.STOT), bool)
    occupied[corefor, slot] = True
    p.call_reg = []
    for _, coff, cn in p.calls:
        s0, s1 = coff * 128, (coff + cn) * 128
        tail = s1 - s0
        for k in range(NCORES):
            t = 0
            while t < s1 - s0 and not occupied[k, s1 - 1 - t]:
                t += 1
            tail = min(tail, t)
        tail = min(tail, s1 - s0 - 1)
        # trimming disabled: skipped slots leave uninitialized SBUF in the
        # matmul lhsT, and NaN*0 = NaN corrupts the accumulation. Re-enable
        # only together with a gt-tile memset.
        tail = 0
        if tail > 0:
            idx_all[:, s1 - tail : s1] = -1
        p.call_reg.append((s1 - s0) - tail)

    # dma_gather index layout: [128, STOT/16] int16, slot s at [s%16, s//16],
    # replicated across the 8 groups of 16 partitions
    idx16 = idx_all.astype(np.int16).reshape(NCORES, p.STOT // 16, 16)
    idx16 = np.ascontiguousarray(idx16.transpose(0, 2, 1))
    p.idx16 = np.ascontiguousarray(np.tile(idx16, (1, 8, 1)))
    # per-chunk dst_local metadata, [128, NCH] with column = chunk
    p.dl = np.ascontiguousarray(dl_all.reshape(NCORES, p.NCH, 128).transpose(0, 2, 1))

    p.has_b1 = None  # set in make_in_maps; program structure depends on it
    p.key = None
    return p


def make_in_maps(plan, x, W1, b1, W2, b2):
    p = plan
    N = p.N
    b1 = np.asarray(b1, np.float32)
    b2 = np.asarray(b2, np.float32)
    p.has_b1 = bool(np.any(b1 != 0.0))
    p.key = (p.NPAD, p.B, tuple(p.chl), tuple(p.chh), p.has_b1)

    dis = p.dis  # original-id order, [NPAD]
    xpad = np.zeros((p.NPAD, D), np.float32)
    xpad[:N] = x

    # layer-1 gather table: row newid[n] = x[n]*dis[n]
    xd = np.zeros((p.NPAD, D), np.float32)
    xd[p.newid] = xpad * dis[:, None]
    xd = xd.astype(ml_dtypes.bfloat16)

    iota = np.tile(np.arange(128, dtype=np.float32)[None, :], (128, 1))

    common = {
        "xd": xd,
        "w1t": np.ascontiguousarray(np.asarray(W1, np.float32).T).astype(
            ml_dtypes.bfloat16
        ),
        "w2t": np.ascontiguousarray(np.asarray(W2, np.float32).T).astype(
            ml_dtypes.bfloat16
        ),
        "iota": iota.astype(ml_dtypes.bfloat16),
        "b2col": np.ascontiguousarray(b2.reshape(128, 1)),
    }

    maps = []
    for k in range(NCORES):
        cols = np.arange(k * p.PCN, (k + 1) * p.PCN)
        orig = p.node_at[cols]  # original node id per local column
        dcol = dis[orig].astype(np.float32)  # dis per local dst column
        xTk = xpad[orig].T  # [128(c), PCN]
        m = dict(
            common,
            idx=p.idx16[k],
            dln=p.dl[k],
            # self-loop term pre-W: x[d]*dis[d] (the dst-side dis scale is
            # applied after the W matmul, completing the dis^2 self norm)
            xd2selfT=np.ascontiguousarray(
                (xTk * dcol[None, :]).astype(ml_dtypes.bfloat16)
            ),
            disoT=np.ascontiguousarray(
                np.tile(dcol[None, :], (128, 1)).astype(ml_dtypes.bfloat16)
            ),
            disq2T=np.ascontiguousarray(
                np.tile((dcol * dcol)[None, :], (128, 1)).astype(ml_dtypes.bfloat16)
            ),
        )
        if p.has_b1:
            m["biasd1T"] = np.ascontiguousarray(
                (b1[:, None] * dcol[None, :]).astype(ml_dtypes.bfloat16)
            )
        maps.append(m)
    return maps


def build_program(plan):
    p = plan
    assert p.has_b1 is not None, "call make_in_maps before build_program"

    nc = bacc.Bacc(
        "TRN2",
        target_bir_lowering=False,
        debug=False,
        num_devices=NCORES,
        num_swdge_queues=_NQUEUES,
        dynamic_dma_scratch_size=_SCRATCH,
    )

    xd_d = nc.dram_tensor("xd", [p.NPAD, 128], BF16, kind="ExternalInput")
    w1t_d = nc.dram_tensor("w1t", [128, 128], BF16, kind="ExternalInput")
    w2t_d = nc.dram_tensor("w2t", [128, 128], BF16, kind="ExternalInput")
    iota_d = nc.dram_tensor("iota", [128, 128], BF16, kind="ExternalInput")
    idx_d = nc.dram_tensor("idx", [128, p.STOT // 16], I16, kind="ExternalInput")
    dln_d = nc.dram_tensor("dln", [128, p.NCH], F32, kind="ExternalInput")
    xd2selfT_d = nc.dram_tensor("xd2selfT", [128, p.PCN], BF16, kind="ExternalInput")
    disoT_d = nc.dram_tensor("disoT", [128, p.PCN], BF16, kind="ExternalInput")
    disq2T_d = nc.dram_tensor("disq2T", [128, p.PCN], BF16, kind="ExternalInput")
    b2col_d = nc.dram_tensor("b2col", [128, 1], F32, kind="ExternalInput")
    if p.has_b1:
        biasd1T_d = nc.dram_tensor("biasd1T", [128, p.PCN], BF16, kind="ExternalInput")
    out_d = nc.dram_tensor("out", [128, p.PCN], F32, kind="ExternalOutput")

    with tile.TileContext(nc) as tc:
        with (
            tc.tile_pool(name="dram", bufs=1, space="DRAM") as dpool,
            tc.tile_pool(name="const", bufs=1) as cpool,
            tc.tile_pool(name="work", bufs=2) as wpool,
            tc.tile_pool(name="psum", bufs=2, space="PSUM") as pspool,
        ):
            agin_t = dpool.tile([p.PCN, 128], BF16, name="aginbuf")
            agout_t = dpool.tile([p.NPAD, 128], BF16, name="agoutbuf")

            def cload(dram, shape, dtype, name):
                t = cpool.tile(shape, dtype, name=name)
                nc.sync.dma_start(out=t[:], in_=dram.ap())
                return t

            w1t_s = cload(w1t_d, [128, 128], BF16, "w1t_s")
            w2t_s = cload(w2t_d, [128, 128], BF16, "w2t_s")
            iota_s = cload(iota_d, [128, 128], BF16, "iota_s")
            idx_s = cload(idx_d, [128, p.STOT // 16], I16, "idx_s")
            dln_s = cload(dln_d, [128, p.NCH], F32, "dln_s")
            xd2selfT_s = cload(xd2selfT_d, [128, p.PCN], BF16, "xd2selfT_s")
            disoT_s = cload(disoT_d, [128, p.PCN], BF16, "disoT_s")
            disq2T_s = cload(disq2T_d, [128, p.PCN], BF16, "disq2T_s")
            b2col_s = cload(b2col_d, [128, 1], F32, "b2col_s")
            if p.has_b1:
                biasd1T_s = cload(biasd1T_d, [128, p.PCN], BF16, "biasd1T_s")

            # layer-1 epilogue outputs, consumed later (single-buf staging)
            a1rows = cpool.tile([128, p.B, 128], BF16, name="a1rows")
            selfnextT = cpool.tile([128, p.PCN], BF16, name="selfnextT")
            outst = cpool.tile([128, p.B, 128], F32, name="outst")

            def emit_layer(lyr):
                wst = w1t_s if lyr == 1 else w2t_s
                tab = xd_d.ap() if lyr == 1 else agout_t[:, :]
                tablo = tab[0 : p.HALF, :]
                tabhi = tab[p.HALF : p.NPAD, :]

                # all gather calls up front (queue-pipelined)
                gts = []
                for gi, (is_h, coff, cn) in enumerate(p.calls):
                    gt = wpool.tile(
                        [128, GCMAX, 128],
                        BF16,
                        tag=("gtH" if is_h else "gtL"),
                        bufs=_GBUFS,
                        name="gt",
                    )
                    ni = cn * 128
                    soff = coff * 128
                    nc.gpsimd.dma_gather(
                        gt[:, :cn, :],
                        tabhi if is_h else tablo,
                        idx_s[:, soff // 16 : (soff + ni) // 16],
                        ni,
                        p.call_reg[gi],
                        128,
                        elem_step=128,
                        single_packet=(ni <= 1024),
                        queue_num=gi % _NQUEUES,
                    )
                    gts.append(gt)

                ptgs = {}

                def get_ptg(gi):
                    t = ptgs.get(gi)
                    if t is None:
                        is_h, coff, cn = p.calls[gi]
                        t = wpool.tile(
                            [128, GCMAX, 128],
                            BF16,
                            tag=("ptH" if is_h else "ptL"),
                            bufs=_PTBUFS,
                            name="ptg",
                        )
                        nc.vector.tensor_tensor(
                            out=t[:, :cn, :],
                            in0=iota_s[:, None, :].to_broadcast((128, cn, 128)),
                            in1=dln_s[:, coff : coff + cn, None].to_broadcast(
                                (128, cn, 128)
                            ),
                            op=mybir.AluOpType.is_equal,
                        )
                        ptgs[gi] = t
                    return t

                for b in range(p.B):
                    cs = slice(b * 128, (b + 1) * 128)
                    agg = pspool.tile([128, 128], F32, tag="agg", bufs=4, name="agg")
                    chunk_ids = [p.lofs[b] + c for c in range(p.chl[b])] + [
                        p.hofs[b] + c for c in range(p.chh[b])
                    ]
                    nch = len(chunk_ids)
                    for k, ci in enumerate(chunk_ids):
                        gi, c = p.chunk_call[ci]
                        ptg = get_ptg(gi)
                        _, coff, _ = p.calls[gi]
                        nc.tensor.matmul(
                            out=agg[:],
                            lhsT=gts[gi][:, c, :],
                            rhs=ptg[:, ci - coff, :],
                            start=(k == 0),
                            stop=(k == nch - 1),
                        )
                    # self-loop folded into the PSUM->SBUF copy
                    u = wpool.tile([128, 128], BF16, tag="u", bufs=4, name="u")
                    selftab = xd2selfT_s if lyr == 1 else selfnextT
                    nc.vector.tensor_tensor(
                        out=u[:], in0=agg[:], in1=selftab[:, cs], op=mybir.AluOpType.add
                    )
                    v = pspool.tile([128, 128], F32, tag="v", bufs=4, name="v")
                    nc.tensor.matmul(
                        out=v[:], lhsT=wst[:], rhs=u[:], start=True, stop=True
                    )
                    if lyr == 1:
                        # zd = v*dis^2 (+ b1*dis); tabT = lrelu(zd) = a1*dis
                        zd = wpool.tile([128, 128], F32, tag="zd", bufs=3, name="zd")
                        nc.vector.tensor_tensor(
                            out=zd[:],
                            in0=v[:],
                            in1=disq2T_s[:, cs],
                            op=mybir.AluOpType.mult,
                        )
                        if p.has_b1:
                            zd2 = wpool.tile(
                                [128, 128], F32, tag="zd2", bufs=3, name="zd2"
                            )
                            nc.vector.tensor_tensor(
                                out=zd2[:],
                                in0=zd[:],
                                in1=biasd1T_s[:, cs],
                                op=mybir.AluOpType.add,
                            )
                            zd = zd2
                        t3 = wpool.tile([128, 128], F32, tag="t3", bufs=3, name="t3")
                        nc.vector.tensor_scalar(
                            t3[:], zd[:], NEG_SLOPE, None, mybir.AluOpType.mult
                        )
                        # tabT = lrelu(zd) = a1*dis: both the layer-2 table
                        # value AND the layer-2 self term -> write directly
                        # into the persistent selfnextT staging
                        nc.vector.tensor_tensor(
                            out=selfnextT[:, cs],
                            in0=zd[:],
                            in1=t3[:],
                            op=mybir.AluOpType.max,
                        )
                        # row-major staging for the AllGather input
                        nc.sync.dma_start_transpose(
                            out=a1rows[:, b, :], in_=selfnextT[:, cs]
                        )
                    else:
                        t = wpool.tile([128, 128], F32, tag="t", bufs=3, name="t")
                        nc.vector.tensor_tensor(
                            out=t[:],
                            in0=v[:],
                            in1=disoT_s[:, cs],
                            op=mybir.AluOpType.mult,
                        )
                        nc.vector.tensor_scalar(
                            outst[:, b, :],
                            t[:],
                            b2col_s[:, 0:1],
                            None,
                            mybir.AluOpType.add,
                        )

            emit_layer(1)
            nc.sync.dma_start(
                out=agin_t[:, :].rearrange("(b q) f -> q b f", q=128),
                in_=a1rows[:, :, :],
            )
            nc.gpsimd.collective_compute(
                "AllGather",
                mybir.AluOpType.bypass,
                replica_groups=[list(range(NCORES))],
                ins=[agin_t[:, :].opt()],
                outs=[agout_t[:, :].opt()],
            )
            emit_layer(2)
            nc.sync.dma_start(out=out_d.ap(), in_=outst[:, :, :])

    nc.compile()
    return nc


_CACHE = {}


def _get_program(plan):
    nc = _CACHE.get(plan.key)
    if nc is None:
        nc = build_program(plan)
        _CACHE[plan.key] = nc
    return nc


def kernel(x, edge_index, batch, W1, b1, W2, b2):
    from concourse.bass_utils import run_bass_kernel_spmd

    x = np.asarray(x, np.float32)
    edge_index = np.asarray(edge_index)
    plan = make_plan(x.shape[0], edge_index)
    in_maps = make_in_maps(
        plan,
        x,
        np.asarray(W1, np.float32),
        np.asarray(b1, np.float32),
        np.asarray(W2, np.float32),
        np.asarray(b2, np.float32),
    )
    nc = _get_program(plan)
    res = run_bass_kernel_spmd(nc, in_maps, core_ids=list(range(NCORES)))
    big = np.concatenate(
        [res.results[k]["out"] for k in range(NCORES)], axis=1
    )  # [128, NPAD]
    out = big[:, plan.newid[: plan.N]].T
    return np.ascontiguousarray(out).astype(np.float32)


# revision 12
# speedup vs baseline: 1.8897x; 1.3649x over previous
"""2-layer GCN (GCNConv -> LeakyReLU -> GCNConv) on 8 Trainium2 NeuronCores.

v2: aggregate-then-transform. GCN's aggregation commutes with the linear map
(A_norm @ (X W) == (A_norm @ X) W), so each layer gathers RAW (pre-scaled)
node features and applies W once per 128-dst block afterwards:

  - dst-partition the graph across 8 cores; host ships xd = x*dis (bf16 rows)
    as the layer-1 gather table, so layer-1 gathers start immediately (no
    on-device dense phase before them).
  - per dst block: aggXT[c,d] = sum_slots gathered[slot,c]*onehot[slot,d]
    accumulated in PSUM via one matmul per 128-slot chunk (lhsT = gathered
    tile, rhs = one-hot). One-hot tiles are built in batches of GCMAX chunks
    with a single broadcast is_equal per gather call.
  - self-loops are folded analytically: u = aggXT + (x*dis^2)[:,block] in the
    PSUM->SBUF copy (one tensor_tensor add); no self-loop gather slots.
  - v = W.T-matmul (stationary w per layer); epilogue applies the dst-side
    dis scale, bias, and leaky-relu in feature-major space; layer-1 output
    rows (a1*dis, the layer-2 table values) are produced via one 128x128
    SBUF->SBUF DMA-transpose per block into a row-major staging tile.
  - a1 rows -> AllGather([PCN,128] -> [NPAD,128]); the collective output IS
    the layer-2 gather table (no rebuild).
  - layer-2 output stays feature-major; the host transposes and un-permutes.

Nodes are relabeled on the host (snake assignment over degree-sorted nodes)
so per-(core,block,half) edge counts are balanced: the chunk schedule is
shared across cores, so padding is set by the max count - balancing makes
max ~= mean and cuts ~13% of gather slots vs. naive labeling.
"""

import math
import os as _os

import numpy as np
import ml_dtypes

from concourse import bacc, bass, mybir
import concourse.tile as tile

BF16 = mybir.dt.bfloat16
F32 = mybir.dt.float32
I16 = mybir.dt.int16

NCORES = 8
D = 128
NEG_SLOPE = 0.01
GCMAX = int(_os.environ.get("GCN_GCMAX", "12"))  # chunks per dma_gather call
_NQUEUES = int(_os.environ.get("GCN_NQUEUES", "4"))
_GBUFS = int(_os.environ.get("GCN_GBUFS", "5"))  # gather tile bufs per region
_PTBUFS = int(_os.environ.get("GCN_PTBUFS", "3"))
_SCRATCH = int(_os.environ.get("GCN_SCRATCH", "24576"))


class Plan:
    pass


def make_plan(n_nodes, edge_index):
    p = Plan()
    src = edge_index[0].astype(np.int64)
    dst = edge_index[1].astype(np.int64)

    unit = NCORES * 128
    p.N = n_nodes
    p.NPAD = ((n_nodes + unit - 1) // unit) * unit
    p.PCN = p.NPAD // NCORES
    p.B = p.PCN // 128
    p.NB = p.NPAD // NCORES * NCORES // 128
    p.NB = p.NPAD // 128
    p.HALF = p.NPAD // 2
    assert p.HALF - 1 <= 32767, "node count too large for int16 half-split"

    deg = np.bincount(dst, minlength=p.NPAD).astype(np.float64) + 1.0
    dis = (1.0 / np.sqrt(deg)).astype(np.float32)
    p.dis = dis

    # snake relabeling: sort nodes by in-degree, deal one per bin per round
    # (alternating direction) -> every 128-node block has ~equal total degree
    order = np.argsort(-deg, kind="stable")
    arr = order.reshape(128, p.NB).copy()
    arr[1::2] = arr[1::2, ::-1]
    newid = np.empty(p.NPAD, np.int64)
    newid[arr] = (np.arange(p.NB)[None, :] * 128 + np.arange(128)[:, None])
    node_at = np.empty(p.NPAD, np.int64)
    node_at[newid] = np.arange(p.NPAD)
    p.newid = newid
    p.node_at = node_at

    src_n = newid[src]
    dst_n = newid[dst]

    core = dst_n // p.PCN
    lb = (dst_n % p.PCN) // 128
    dloc = (dst_n % 128).astype(np.float32)
    halfbit = (src_n >= p.HALF).astype(np.int64)
    seg = (core * p.B + lb) * 2 + halfbit
    nseg = NCORES * p.B * 2

    sorder = np.lexsort((src_n, seg))
    seg_s = seg[sorder]
    src_s = src_n[sorder]
    dloc_s = dloc[sorder]

    counts = np.bincount(seg_s, minlength=nseg)
    cnt = counts.reshape(NCORES, p.B, 2)
    p.chl = [max(1, int(math.ceil(cnt[:, b, 0].max() / 128))) for b in range(p.B)]
    p.chh = [max(1, int(math.ceil(cnt[:, b, 1].max() / 128))) for b in range(p.B)]
    p.SLch = sum(p.chl)
    p.SHch = sum(p.chh)
    p.NCH = p.SLch + p.SHch
    p.STOT = p.NCH * 128
    p.lofs = np.concatenate([[0], np.cumsum(p.chl)])[:-1]
    p.hofs = p.SLch + np.concatenate([[0], np.cumsum(p.chh)])[:-1]

    segid = np.arange(nseg)
    sblk = (segid // 2) % p.B
    sh = segid % 2
    base = np.where(sh == 0, p.lofs[sblk] * 128, p.hofs[sblk] * 128)

    seg_starts = np.zeros(nseg + 1, np.int64)
    np.cumsum(counts, out=seg_starts[1:])
    rank = np.arange(len(seg_s)) - seg_starts[seg_s]
    slot = base[seg_s] + rank
    corefor = seg_s // (2 * p.B)

    idx_all = np.zeros((NCORES, p.STOT), np.int32)
    # pad slots: idx 0 (safe row), dst_local -1 so is_equal zeroes the column
    dl_all = np.full((NCORES, p.STOT), -1.0, np.float32)
    val = np.where(src_s >= p.HALF, src_s - p.HALF, src_s)
    idx_all[corefor, slot] = val
    dl_all[corefor, slot] = dloc_s

    # gather call plan: (is_h, chunk_off, nchunks), GCMAX chunks per call
    p.calls = []
    for is_h, n_region, off in ((0, p.SLch, 0), (1, p.SHch, p.SLch)):
        nc_calls = max(1, math.ceil(n_region / GCMAX))
        per = math.ceil(n_region / nc_calls)
        c0 = 0
        while c0 < n_region:
            cn = min(per, n_region - c0)
            p.calls.append((is_h, off + c0, cn))
            c0 += cn
    p.chunk_call = np.zeros((p.NCH, 2), np.int64)
    for gi, (_, coff, cn) in enumerate(p.calls):
        for c in range(cn):
            p.chunk_call[coff + c] = (gi, c)

    # mark pad slots at each call's tail as -1: the Q7 kernel trims trailing
    # negatives, skipping their descriptor generation + DMA entirely. The
    # num_idxs register is a shared program constant, so only the tail run
    # that is padding on EVERY core can be trimmed (mid-stream negatives are
    # NOT safe - uint32 address math - tails only).
    occupied = np.zeros((NCORES, p.STOT), bool)
    occupied[corefor, slot] = True
    p.call_reg = []
    for _, coff, cn in p.calls:
        s0, s1 = coff * 128, (coff + cn) * 128
        tail = s1 - s0
        for k in range(NCORES):
            t = 0
            while t < s1 - s0 and not occupied[k, s1 - 1 - t]:
                t += 1
            tail = min(tail, t)
        tail = min(tail, s1 - s0 - 1)
        # trimming disabled: skipped slots leave uninitialized SBUF in the
        # matmul lhsT, and NaN*0 = NaN corrupts the accumulation. Re-enable
        # only together with a gt-tile memset.
        tail = 0
        if tail > 0:
            idx_all[:, s1 - tail : s1] = -1
        p.call_reg.append((s1 - s0) - tail)

    # dma_gather index layout: [128, STOT/16] int16, slot s at [s%16, s//16],
    # replicated across the 8 groups of 16 partitions
    idx16 = idx_all.astype(np.int16).reshape(NCORES, p.STOT // 16, 16)
    idx16 = np.ascontiguousarray(idx16.transpose(0, 2, 1))
    p.idx16 = np.ascontiguousarray(np.tile(idx16, (1, 8, 1)))
    # per-chunk dst_local metadata, [128, NCH] with column = chunk
    p.dl = np.ascontiguousarray(dl_all.reshape(NCORES, p.NCH, 128).transpose(0, 2, 1))

    p.has_b1 = None  # set in make_in_maps; program structure depends on it
    p.key = None
    return p


def make_in_maps(plan, x, W1, b1, W2, b2):
    p = plan
    N = p.N
    b1 = np.asarray(b1, np.float32)
    b2 = np.asarray(b2, np.float32)
    p.has_b1 = bool(np.any(b1 != 0.0))
    p.key = (p.NPAD, p.B, tuple(p.chl), tuple(p.chh), p.has_b1)

    dis = p.dis  # original-id order, [NPAD]
    xpad = np.zeros((p.NPAD, D), np.float32)
    xpad[:N] = x

    # layer-1 gather table: row newid[n] = x[n]*dis[n]
    xd = np.zeros((p.NPAD, D), np.float32)
    xd[p.newid] = xpad * dis[:, None]
    xd = xd.astype(ml_dtypes.bfloat16)

    iota = np.tile(np.arange(128, dtype=np.float32)[None, :], (128, 1))

    common = {
        "xd": xd,
        "w1t": np.ascontiguousarray(np.asarray(W1, np.float32).T).astype(
            ml_dtypes.bfloat16
        ),
        "w2t": np.ascontiguousarray(np.asarray(W2, np.float32).T).astype(
            ml_dtypes.bfloat16
        ),
        "iota": iota.astype(ml_dtypes.bfloat16),
        "ident": np.eye(128, dtype=np.float32).astype(ml_dtypes.bfloat16),
        "b2col": np.ascontiguousarray(b2.reshape(128, 1)),
    }

    maps = []
    for k in range(NCORES):
        cols = np.arange(k * p.PCN, (k + 1) * p.PCN)
        orig = p.node_at[cols]  # original node id per local column
        dcol = dis[orig].astype(np.float32)  # dis per local dst column
        xTk = xpad[orig].T  # [128(c), PCN]
        m = dict(
            common,
            idx=p.idx16[k],
            dln=p.dl[k],
            # self-loop term pre-W: x[d]*dis[d] (the dst-side dis scale is
            # applied after the W matmul, completing the dis^2 self norm)
            xd2selfT=np.ascontiguousarray(
                (xTk * dcol[None, :]).astype(ml_dtypes.bfloat16)
            ),
            disoT=np.ascontiguousarray(
                np.tile(dcol[None, :], (128, 1)).astype(ml_dtypes.bfloat16)
            ),
            disq2T=np.ascontiguousarray(
                np.tile((dcol * dcol)[None, :], (128, 1)).astype(ml_dtypes.bfloat16)
            ),
        )
        if p.has_b1:
            m["biasd1T"] = np.ascontiguousarray(
                (b1[:, None] * dcol[None, :]).astype(ml_dtypes.bfloat16)
            )
        maps.append(m)
    return maps


def build_program(plan):
    p = plan
    assert p.has_b1 is not None, "call make_in_maps before build_program"

    nc = bacc.Bacc(
        "TRN2",
        target_bir_lowering=False,
        debug=False,
        num_devices=NCORES,
        num_swdge_queues=_NQUEUES,
        dynamic_dma_scratch_size=_SCRATCH,
    )

    xd_d = nc.dram_tensor("xd", [p.NPAD, 128], BF16, kind="ExternalInput")
    w1t_d = nc.dram_tensor("w1t", [128, 128], BF16, kind="ExternalInput")
    w2t_d = nc.dram_tensor("w2t", [128, 128], BF16, kind="ExternalInput")
    iota_d = nc.dram_tensor("iota", [128, 128], BF16, kind="ExternalInput")
    ident_d = nc.dram_tensor("ident", [128, 128], BF16, kind="ExternalInput")
    idx_d = nc.dram_tensor("idx", [128, p.STOT // 16], I16, kind="ExternalInput")
    dln_d = nc.dram_tensor("dln", [128, p.NCH], F32, kind="ExternalInput")
    xd2selfT_d = nc.dram_tensor("xd2selfT", [128, p.PCN], BF16, kind="ExternalInput")
    disoT_d = nc.dram_tensor("disoT", [128, p.PCN], BF16, kind="ExternalInput")
    disq2T_d = nc.dram_tensor("disq2T", [128, p.PCN], BF16, kind="ExternalInput")
    b2col_d = nc.dram_tensor("b2col", [128, 1], F32, kind="ExternalInput")
    if p.has_b1:
        biasd1T_d = nc.dram_tensor("biasd1T", [128, p.PCN], BF16, kind="ExternalInput")
    out_d = nc.dram_tensor("out", [128, p.PCN], F32, kind="ExternalOutput")

    with tile.TileContext(nc) as tc:
        with (
            tc.tile_pool(name="dram", bufs=1, space="DRAM") as dpool,
            tc.tile_pool(name="const", bufs=1) as cpool,
            tc.tile_pool(name="work", bufs=2) as wpool,
            tc.tile_pool(name="psum", bufs=2, space="PSUM") as pspool,
        ):
            agin_t = dpool.tile([p.PCN, 128], BF16, name="aginbuf")
            agout_t = dpool.tile([p.NPAD, 128], BF16, name="agoutbuf")

            def cload(dram, shape, dtype, name):
                t = cpool.tile(shape, dtype, name=name)
                nc.sync.dma_start(out=t[:], in_=dram.ap())
                return t

            w1t_s = cload(w1t_d, [128, 128], BF16, "w1t_s")
            w2t_s = cload(w2t_d, [128, 128], BF16, "w2t_s")
            iota_s = cload(iota_d, [128, 128], BF16, "iota_s")
            ident_s = cload(ident_d, [128, 128], BF16, "ident_s")
            idx_s = cload(idx_d, [128, p.STOT // 16], I16, "idx_s")
            dln_s = cload(dln_d, [128, p.NCH], F32, "dln_s")
            xd2selfT_s = cload(xd2selfT_d, [128, p.PCN], BF16, "xd2selfT_s")
            disoT_s = cload(disoT_d, [128, p.PCN], BF16, "disoT_s")
            disq2T_s = cload(disq2T_d, [128, p.PCN], BF16, "disq2T_s")
            b2col_s = cload(b2col_d, [128, 1], F32, "b2col_s")
            if p.has_b1:
                biasd1T_s = cload(biasd1T_d, [128, p.PCN], BF16, "biasd1T_s")

            # layer-1 epilogue outputs, consumed later (single-buf staging)
            a1rows = cpool.tile([128, p.B, 128], BF16, name="a1rows")
            selfnextT = cpool.tile([128, p.PCN], BF16, name="selfnextT")
            outst = cpool.tile([128, p.B, 128], F32, name="outst")

            def emit_layer(lyr):
                wst = w1t_s if lyr == 1 else w2t_s
                tab = xd_d.ap() if lyr == 1 else agout_t[:, :]
                tablo = tab[0 : p.HALF, :]
                tabhi = tab[p.HALF : p.NPAD, :]

                # all gather calls up front (queue-pipelined)
                gts = []
                for gi, (is_h, coff, cn) in enumerate(p.calls):
                    gt = wpool.tile(
                        [128, GCMAX, 128],
                        BF16,
                        tag=("gtH" if is_h else "gtL"),
                        bufs=_GBUFS,
                        name="gt",
                    )
                    ni = cn * 128
                    soff = coff * 128
                    nc.gpsimd.dma_gather(
                        gt[:, :cn, :],
                        tabhi if is_h else tablo,
                        idx_s[:, soff // 16 : (soff + ni) // 16],
                        ni,
                        p.call_reg[gi],
                        128,
                        elem_step=128,
                        single_packet=(ni <= 1024),
                        queue_num=gi % _NQUEUES,
                    )
                    gts.append(gt)

                ptgs = {}

                def get_ptg(gi):
                    t = ptgs.get(gi)
                    if t is None:
                        is_h, coff, cn = p.calls[gi]
                        t = wpool.tile(
                            [128, GCMAX, 128],
                            BF16,
                            tag=("ptH" if is_h else "ptL"),
                            bufs=_PTBUFS,
                            name="ptg",
                        )
                        nc.vector.tensor_tensor(
                            out=t[:, :cn, :],
                            in0=iota_s[:, None, :].to_broadcast((128, cn, 128)),
                            in1=dln_s[:, coff : coff + cn, None].to_broadcast(
                                (128, cn, 128)
                            ),
                            op=mybir.AluOpType.is_equal,
                        )
                        ptgs[gi] = t
                    return t

                for b in range(p.B):
                    cs = slice(b * 128, (b + 1) * 128)
                    agg = pspool.tile([128, 128], F32, tag="agg", bufs=3, name="agg")
                    chunk_ids = [p.lofs[b] + c for c in range(p.chl[b])] + [
                        p.hofs[b] + c for c in range(p.chh[b])
                    ]
                    nch = len(chunk_ids)
                    for k, ci in enumerate(chunk_ids):
                        gi, c = p.chunk_call[ci]
                        ptg = get_ptg(gi)
                        _, coff, _ = p.calls[gi]
                        nc.tensor.matmul(
                            out=agg[:],
                            lhsT=gts[gi][:, c, :],
                            rhs=ptg[:, ci - coff, :],
                            start=(k == 0),
                            stop=(k == nch - 1),
                        )
                    # self-loop folded into the PSUM->SBUF copy
                    u = wpool.tile([128, 128], BF16, tag="u", bufs=4, name="u")
                    selftab = xd2selfT_s if lyr == 1 else selfnextT
                    nc.vector.tensor_tensor(
                        out=u[:], in0=agg[:], in1=selftab[:, cs], op=mybir.AluOpType.add
                    )
                    v = pspool.tile([128, 128], F32, tag="v", bufs=3, name="v")
                    nc.tensor.matmul(
                        out=v[:], lhsT=wst[:], rhs=u[:], start=True, stop=True
                    )
                    if lyr == 1:
                        # zd = v*dis^2 (+ b1*dis); tabT = lrelu(zd) = a1*dis
                        zd = wpool.tile([128, 128], F32, tag="zd", bufs=3, name="zd")
                        nc.vector.tensor_tensor(
                            out=zd[:],
                            in0=v[:],
                            in1=disq2T_s[:, cs],
                            op=mybir.AluOpType.mult,
                        )
                        if p.has_b1:
                            zd2 = wpool.tile(
                                [128, 128], F32, tag="zd2", bufs=3, name="zd2"
                            )
                            nc.vector.tensor_tensor(
                                out=zd2[:],
                                in0=zd[:],
                                in1=biasd1T_s[:, cs],
                                op=mybir.AluOpType.add,
                            )
                            zd = zd2
                        t3 = wpool.tile([128, 128], F32, tag="t3", bufs=3, name="t3")
                        nc.vector.tensor_scalar(
                            t3[:], zd[:], NEG_SLOPE, None, mybir.AluOpType.mult
                        )
                        # tabT = lrelu(zd) = a1*dis: both the layer-2 table
                        # value AND the layer-2 self term -> write directly
                        # into the persistent selfnextT staging
                        nc.vector.tensor_tensor(
                            out=selfnextT[:, cs],
                            in0=zd[:],
                            in1=t3[:],
                            op=mybir.AluOpType.max,
                        )
                        # row-major staging for the AllGather input via PE
                        # transpose (XBAR DMA-transpose serializes against
                        # in-flight SWDGE gathers - measured ~2x layer-1 cost)
                        tp = pspool.tile(
                            [128, 128], BF16, space="PSUM", tag="tp", bufs=2,
                            name="tp",
                        )
                        nc.tensor.transpose(
                            out=tp[:], in_=selfnextT[:, cs], identity=ident_s[:]
                        )
                        nc.vector.tensor_copy(out=a1rows[:, b, :], in_=tp[:])
                    else:
                        t = wpool.tile([128, 128], F32, tag="t", bufs=3, name="t")
                        nc.vector.tensor_tensor(
                            out=t[:],
                            in0=v[:],
                            in1=disoT_s[:, cs],
                            op=mybir.AluOpType.mult,
                        )
                        nc.vector.tensor_scalar(
                            outst[:, b, :],
                            t[:],
                            b2col_s[:, 0:1],
                            None,
                            mybir.AluOpType.add,
                        )

            emit_layer(1)
            nc.sync.dma_start(
                out=agin_t[:, :].rearrange("(b q) f -> q b f", q=128),
                in_=a1rows[:, :, :],
            )
            nc.gpsimd.collective_compute(
                "AllGather",
                mybir.AluOpType.bypass,
                replica_groups=[list(range(NCORES))],
                ins=[agin_t[:, :].opt()],
                outs=[agout_t[:, :].opt()],
            )
            emit_layer(2)
            nc.sync.dma_start(out=out_d.ap(), in_=outst[:, :, :])

    nc.compile()
    return nc


_CACHE = {}


def _get_program(plan):
    nc = _CACHE.get(plan.key)
    if nc is None:
        nc = build_program(plan)
        _CACHE[plan.key] = nc
    return nc


def kernel(x, edge_index, batch, W1, b1, W2, b2):
    from concourse.bass_utils import run_bass_kernel_spmd

    x = np.asarray(x, np.float32)
    edge_index = np.asarray(edge_index)
    plan = make_plan(x.shape[0], edge_index)
    in_maps = make_in_maps(
        plan,
        x,
        np.asarray(W1, np.float32),
        np.asarray(b1, np.float32),
        np.asarray(W2, np.float32),
        np.asarray(b2, np.float32),
    )
    nc = _get_program(plan)
    res = run_bass_kernel_spmd(nc, in_maps, core_ids=list(range(NCORES)))
    big = np.concatenate(
        [res.results[k]["out"] for k in range(NCORES)], axis=1
    )  # [128, NPAD]
    out = big[:, plan.newid[: plan.N]].T
    return np.ascontiguousarray(out).astype(np.float32)


# revision 29
# speedup vs baseline: 2.5550x; 1.3520x over previous
"""2-layer GCN (GCNConv -> LeakyReLU -> GCNConv) on 8 Trainium2 NeuronCores.

v2: aggregate-then-transform. GCN's aggregation commutes with the linear map
(A_norm @ (X W) == (A_norm @ X) W), so each layer gathers RAW (pre-scaled)
node features and applies W once per 128-dst block afterwards:

  - dst-partition the graph across 8 cores; host ships xd = x*dis (bf16 rows)
    as the layer-1 gather table, so layer-1 gathers start immediately (no
    on-device dense phase before them).
  - per dst block: aggXT[c,d] = sum_slots gathered[slot,c]*onehot[slot,d]
    accumulated in PSUM via one matmul per 128-slot chunk (lhsT = gathered
    tile, rhs = one-hot). One-hot tiles are built in batches of GCMAX chunks
    with a single broadcast is_equal per gather call.
  - self-loops are folded analytically: u = aggXT + (x*dis^2)[:,block] in the
    PSUM->SBUF copy (one tensor_tensor add); no self-loop gather slots.
  - v = W.T-matmul (stationary w per layer); epilogue applies the dst-side
    dis scale, bias, and leaky-relu in feature-major space; layer-1 output
    rows (a1*dis, the layer-2 table values) are produced via one 128x128
    SBUF->SBUF DMA-transpose per block into a row-major staging tile.
  - a1 rows -> AllGather([PCN,128] -> [NPAD,128]); the collective output IS
    the layer-2 gather table (no rebuild).
  - layer-2 output stays feature-major; the host transposes and un-permutes.

Nodes are relabeled on the host (snake assignment over degree-sorted nodes)
so per-(core,block,half) edge counts are balanced: the chunk schedule is
shared across cores, so padding is set by the max count - balancing makes
max ~= mean and cuts ~13% of gather slots vs. naive labeling.
"""

import math
import os as _os

import numpy as np
import ml_dtypes

from concourse import bacc, bass, mybir
import concourse.tile as tile

BF16 = mybir.dt.bfloat16
F32 = mybir.dt.float32
I16 = mybir.dt.int16

NCORES = 8
D = 128
NEG_SLOPE = 0.01
GCMAX = int(_os.environ.get("GCN_GCMAX", "12"))  # chunks per dma_gather call
_NQUEUES = int(_os.environ.get("GCN_NQUEUES", "4"))
_GBUFS = int(_os.environ.get("GCN_GBUFS", "5"))  # gather tile bufs per region
_PTBUFS = int(_os.environ.get("GCN_PTBUFS", "3"))
_SCRATCH = int(_os.environ.get("GCN_SCRATCH", "32768"))
# L2 gather calls pre-generated (prepare_only) before/during the AllGather so
# Q7 descriptor generation fills the otherwise-idle collective window.
# W1 preps go before the collective instruction, W2 after (they generate
# while the CC cores run the AllGather). Bounded by SWDGE ring capacity.
_PREPW1 = int(_os.environ.get("GCN_PREPW1", "9"))
_PREPW2 = int(_os.environ.get("GCN_PREPW2", "9"))


class Plan:
    pass


def make_plan(n_nodes, edge_index):
    p = Plan()
    src = edge_index[0].astype(np.int64)
    dst = edge_index[1].astype(np.int64)

    unit = NCORES * 128
    p.N = n_nodes
    p.NPAD = ((n_nodes + unit - 1) // unit) * unit
    p.PCN = p.NPAD // NCORES
    p.B = p.PCN // 128
    p.NB = p.NPAD // NCORES * NCORES // 128
    p.NB = p.NPAD // 128
    p.HALF = p.NPAD // 2
    assert p.HALF - 1 <= 32767, "node count too large for int16 half-split"

    deg = np.bincount(dst, minlength=p.NPAD).astype(np.float64) + 1.0
    dis = (1.0 / np.sqrt(deg)).astype(np.float32)
    p.dis = dis

    # snake relabeling: sort nodes by in-degree, deal one per bin per round
    # (alternating direction) -> every 128-node block has ~equal total degree
    order = np.argsort(-deg, kind="stable")
    arr = order.reshape(128, p.NB).copy()
    arr[1::2] = arr[1::2, ::-1]
    newid = np.empty(p.NPAD, np.int64)
    newid[arr] = (np.arange(p.NB)[None, :] * 128 + np.arange(128)[:, None])
    node_at = np.empty(p.NPAD, np.int64)
    node_at[newid] = np.arange(p.NPAD)
    p.newid = newid
    p.node_at = node_at

    src_n = newid[src]
    dst_n = newid[dst]

    core = dst_n // p.PCN
    lb = (dst_n % p.PCN) // 128
    dloc = (dst_n % 128).astype(np.float32)
    halfbit = (src_n >= p.HALF).astype(np.int64)
    seg = (core * p.B + lb) * 2 + halfbit
    nseg = NCORES * p.B * 2

    sorder = np.lexsort((src_n, seg))
    seg_s = seg[sorder]
    src_s = src_n[sorder]
    dloc_s = dloc[sorder]

    counts = np.bincount(seg_s, minlength=nseg)
    cnt = counts.reshape(NCORES, p.B, 2)
    p.chl = [max(1, int(math.ceil(cnt[:, b, 0].max() / 128))) for b in range(p.B)]
    p.chh = [max(1, int(math.ceil(cnt[:, b, 1].max() / 128))) for b in range(p.B)]
    p.SLch = sum(p.chl)
    p.SHch = sum(p.chh)
    p.NCH = p.SLch + p.SHch
    p.STOT = p.NCH * 128
    p.lofs = np.concatenate([[0], np.cumsum(p.chl)])[:-1]
    p.hofs = p.SLch + np.concatenate([[0], np.cumsum(p.chh)])[:-1]

    segid = np.arange(nseg)
    sblk = (segid // 2) % p.B
    sh = segid % 2
    base = np.where(sh == 0, p.lofs[sblk] * 128, p.hofs[sblk] * 128)

    seg_starts = np.zeros(nseg + 1, np.int64)
    np.cumsum(counts, out=seg_starts[1:])
    rank = np.arange(len(seg_s)) - seg_starts[seg_s]
    slot = base[seg_s] + rank
    corefor = seg_s // (2 * p.B)

    idx_all = np.zeros((NCORES, p.STOT), np.int32)
    # pad slots: idx 0 (safe row), dst_local -1 so is_equal zeroes the column
    dl_all = np.full((NCORES, p.STOT), -1.0, np.float32)
    val = np.where(src_s >= p.HALF, src_s - p.HALF, src_s)
    idx_all[corefor, slot] = val
    dl_all[corefor, slot] = dloc_s

    # gather call plan: (is_h, chunk_off, nchunks), GCMAX chunks per call
    p.calls = []
    for is_h, n_region, off in ((0, p.SLch, 0), (1, p.SHch, p.SLch)):
        nc_calls = max(1, math.ceil(n_region / GCMAX))
        per = math.ceil(n_region / nc_calls)
        c0 = 0
        while c0 < n_region:
            cn = min(per, n_region - c0)
            p.calls.append((is_h, off + c0, cn))
            c0 += cn
    p.chunk_call = np.zeros((p.NCH, 2), np.int64)
    for gi, (_, coff, cn) in enumerate(p.calls):
        for c in range(cn):
            p.chunk_call[coff + c] = (gi, c)

    # mark pad slots at each call's tail as -1: the Q7 kernel trims trailing
    # negatives, skipping their descriptor generation + DMA entirely. The
    # num_idxs register is a shared program constant, so only the tail run
    # that is padding on EVERY core can be trimmed (mid-stream negatives are
    # NOT safe - uint32 address math - tails only).
    occupied = np.zeros((NCORES, p.STOT), bool)
    occupied[corefor, slot] = True
    p.call_reg = []
    for _, coff, cn in p.calls:
        s0, s1 = coff * 128, (coff + cn) * 128
        tail = s1 - s0
        for k in range(NCORES):
            t = 0
            while t < s1 - s0 and not occupied[k, s1 - 1 - t]:
                t += 1
            tail = min(tail, t)
        tail = min(tail, s1 - s0 - 1)
        # trimming disabled: skipped slots leave uninitialized SBUF in the
        # matmul lhsT, and NaN*0 = NaN corrupts the accumulation. Re-enable
        # only together with a gt-tile memset.
        tail = 0
        if tail > 0:
            idx_all[:, s1 - tail : s1] = -1
        p.call_reg.append((s1 - s0) - tail)

    # dma_gather index layout: [128, STOT/16] int16, slot s at [s%16, s//16],
    # replicated across the 8 groups of 16 partitions
    idx16 = idx_all.astype(np.int16).reshape(NCORES, p.STOT // 16, 16)
    idx16 = np.ascontiguousarray(idx16.transpose(0, 2, 1))
    p.idx16 = np.ascontiguousarray(np.tile(idx16, (1, 8, 1)))
    # per-chunk dst_local metadata, [128, NCH] with column = chunk
    p.dl = np.ascontiguousarray(dl_all.reshape(NCORES, p.NCH, 128).transpose(0, 2, 1))

    p.has_b1 = None  # set in make_in_maps; program structure depends on it
    p.key = None
    return p


def make_in_maps(plan, x, W1, b1, W2, b2):
    p = plan
    N = p.N
    b1 = np.asarray(b1, np.float32)
    b2 = np.asarray(b2, np.float32)
    p.has_b1 = bool(np.any(b1 != 0.0))
    p.key = (p.NPAD, p.B, tuple(p.chl), tuple(p.chh), p.has_b1)

    dis = p.dis  # original-id order, [NPAD]
    xpad = np.zeros((p.NPAD, D), np.float32)
    xpad[:N] = x

    # layer-1 gather table: row newid[n] = x[n]*dis[n]
    xd = np.zeros((p.NPAD, D), np.float32)
    xd[p.newid] = xpad * dis[:, None]
    xd = xd.astype(ml_dtypes.bfloat16)

    iota = np.tile(np.arange(128, dtype=np.float32)[None, :], (128, 1))

    common = {
        "xd": xd,
        "w1t": np.ascontiguousarray(np.asarray(W1, np.float32).T).astype(
            ml_dtypes.bfloat16
        ),
        "w2t": np.ascontiguousarray(np.asarray(W2, np.float32).T).astype(
            ml_dtypes.bfloat16
        ),
        "iota": iota.astype(ml_dtypes.bfloat16),
        "ident": np.eye(128, dtype=np.float32).astype(ml_dtypes.bfloat16),
        "b2col": np.ascontiguousarray(b2.reshape(128, 1)),
    }

    maps = []
    for k in range(NCORES):
        cols = np.arange(k * p.PCN, (k + 1) * p.PCN)
        orig = p.node_at[cols]  # original node id per local column
        dcol = dis[orig].astype(np.float32)  # dis per local dst column
        xTk = xpad[orig].T  # [128(c), PCN]
        m = dict(
            common,
            idx=p.idx16[k],
            dln=p.dl[k],
            # self-loop term pre-W: x[d]*dis[d] (the dst-side dis scale is
            # applied after the W matmul, completing the dis^2 self norm)
            xd2selfT=np.ascontiguousarray(
                (xTk * dcol[None, :]).astype(ml_dtypes.bfloat16)
            ),
            disoT=np.ascontiguousarray(
                np.tile(dcol[None, :], (128, 1)).astype(ml_dtypes.bfloat16)
            ),
            disq2T=np.ascontiguousarray(
                np.tile((dcol * dcol)[None, :], (128, 1)).astype(ml_dtypes.bfloat16)
            ),
        )
        if p.has_b1:
            m["biasd1T"] = np.ascontiguousarray(
                (b1[:, None] * dcol[None, :]).astype(ml_dtypes.bfloat16)
            )
        maps.append(m)
    return maps


def build_program(plan):
    p = plan
    assert p.has_b1 is not None, "call make_in_maps before build_program"

    nc = bacc.Bacc(
        "TRN2",
        target_bir_lowering=False,
        debug=False,
        num_devices=NCORES,
        num_swdge_queues=_NQUEUES,
        dynamic_dma_scratch_size=_SCRATCH,
    )

    xd_d = nc.dram_tensor("xd", [p.NPAD, 128], BF16, kind="ExternalInput")
    w1t_d = nc.dram_tensor("w1t", [128, 128], BF16, kind="ExternalInput")
    w2t_d = nc.dram_tensor("w2t", [128, 128], BF16, kind="ExternalInput")
    iota_d = nc.dram_tensor("iota", [128, 128], BF16, kind="ExternalInput")
    ident_d = nc.dram_tensor("ident", [128, 128], BF16, kind="ExternalInput")
    idx_d = nc.dram_tensor("idx", [128, p.STOT // 16], I16, kind="ExternalInput")
    dln_d = nc.dram_tensor("dln", [128, p.NCH], F32, kind="ExternalInput")
    xd2selfT_d = nc.dram_tensor("xd2selfT", [128, p.PCN], BF16, kind="ExternalInput")
    disoT_d = nc.dram_tensor("disoT", [128, p.PCN], BF16, kind="ExternalInput")
    disq2T_d = nc.dram_tensor("disq2T", [128, p.PCN], BF16, kind="ExternalInput")
    b2col_d = nc.dram_tensor("b2col", [128, 1], F32, kind="ExternalInput")
    if p.has_b1:
        biasd1T_d = nc.dram_tensor("biasd1T", [128, p.PCN], BF16, kind="ExternalInput")
    out_d = nc.dram_tensor("out", [128, p.PCN], F32, kind="ExternalOutput")

    with tile.TileContext(nc) as tc:
        with (
            tc.tile_pool(name="dram", bufs=1, space="DRAM") as dpool,
            tc.tile_pool(name="const", bufs=1) as cpool,
            tc.tile_pool(name="work", bufs=2) as wpool,
            tc.tile_pool(name="psum", bufs=2, space="PSUM") as pspool,
        ):
            agin_t = dpool.tile([p.PCN, 128], BF16, name="aginbuf")
            agout_t = dpool.tile([p.NPAD, 128], BF16, name="agoutbuf")

            def cload(dram, shape, dtype, name):
                t = cpool.tile(shape, dtype, name=name)
                nc.sync.dma_start(out=t[:], in_=dram.ap())
                return t

            w1t_s = cload(w1t_d, [128, 128], BF16, "w1t_s")
            w2t_s = cload(w2t_d, [128, 128], BF16, "w2t_s")
            iota_s = cload(iota_d, [128, 128], BF16, "iota_s")
            ident_s = cload(ident_d, [128, 128], BF16, "ident_s")
            idx_s = cload(idx_d, [128, p.STOT // 16], I16, "idx_s")
            dln_s = cload(dln_d, [128, p.NCH], F32, "dln_s")
            xd2selfT_s = cload(xd2selfT_d, [128, p.PCN], BF16, "xd2selfT_s")
            disoT_s = cload(disoT_d, [128, p.PCN], BF16, "disoT_s")
            disq2T_s = cload(disq2T_d, [128, p.PCN], BF16, "disq2T_s")
            b2col_s = cload(b2col_d, [128, 1], F32, "b2col_s")
            if p.has_b1:
                biasd1T_s = cload(biasd1T_d, [128, p.PCN], BF16, "biasd1T_s")

            # layer-1 epilogue outputs, consumed later (single-buf staging)
            a1rows = cpool.tile([128, p.B, 128], BF16, name="a1rows")
            selfnextT = cpool.tile([128, p.PCN], BF16, name="selfnextT")
            outst = cpool.tile([128, p.B, 128], F32, name="outst")

            qsems = [nc.alloc_semaphore(f"gsem{q}") for q in range(_NQUEUES)]

            def emit_gather(lyr, gi, prep=False):
                tab = xd_d.ap() if lyr == 1 else agout_t[:, :]
                is_h, coff, cn = p.calls[gi]
                gt = wpool.tile(
                    [128, GCMAX, 128],
                    BF16,
                    tag=("gtH" if is_h else "gtL"),
                    bufs=_GBUFS,
                    name="gt",
                )
                ni = cn * 128
                soff = coff * 128
                q = gi % _NQUEUES
                nc.gpsimd.dma_gather(
                    gt[:, :cn, :],
                    tab[p.HALF : p.NPAD, :] if is_h else tab[0 : p.HALF, :],
                    idx_s[:, soff // 16 : (soff + ni) // 16],
                    ni,
                    p.call_reg[gi],
                    128,
                    elem_step=128,
                    # single-packet rings hold ONE pending entry: a second
                    # untriggered prep on the same queue corrupts the first
                    single_packet=(ni <= 1024) and not prep,
                    queue_num=q,
                    prepare_only=prep,
                    sem=qsems[q] if prep else None,
                )
                return gt

            def emit_layer(lyr, gts):
                wst = w1t_s if lyr == 1 else w2t_s
                # remaining (non-prepped) gather calls for this layer
                for gi in range(len(gts), len(p.calls)):
                    gts.append(emit_gather(lyr, gi))

                ptgs = {}

                def get_ptg(gi):
                    t = ptgs.get(gi)
                    if t is None:
                        is_h, coff, cn = p.calls[gi]
                        t = wpool.tile(
                            [128, GCMAX, 128],
                            BF16,
                            tag=("ptH" if is_h else "ptL"),
                            bufs=_PTBUFS,
                            name="ptg",
                        )
                        nc.vector.tensor_tensor(
                            out=t[:, :cn, :],
                            in0=iota_s[:, None, :].to_broadcast((128, cn, 128)),
                            in1=dln_s[:, coff : coff + cn, None].to_broadcast(
                                (128, cn, 128)
                            ),
                            op=mybir.AluOpType.is_equal,
                        )
                        ptgs[gi] = t
                    return t

                for b in range(p.B):
                    cs = slice(b * 128, (b + 1) * 128)
                    agg = pspool.tile([128, 128], F32, tag="agg", bufs=3, name="agg")
                    chunk_ids = [p.lofs[b] + c for c in range(p.chl[b])] + [
                        p.hofs[b] + c for c in range(p.chh[b])
                    ]
                    nch = len(chunk_ids)
                    for k, ci in enumerate(chunk_ids):
                        gi, c = p.chunk_call[ci]
                        ptg = get_ptg(gi)
                        _, coff, _ = p.calls[gi]
                        nc.tensor.matmul(
                            out=agg[:],
                            lhsT=gts[gi][:, c, :],
                            rhs=ptg[:, ci - coff, :],
                            start=(k == 0),
                            stop=(k == nch - 1),
                        )
                    # self-loop folded into the PSUM->SBUF copy
                    u = wpool.tile([128, 128], BF16, tag="u", bufs=4, name="u")
                    selftab = xd2selfT_s if lyr == 1 else selfnextT
                    nc.vector.tensor_tensor(
                        out=u[:], in0=agg[:], in1=selftab[:, cs], op=mybir.AluOpType.add
                    )
                    v = pspool.tile([128, 128], F32, tag="v", bufs=3, name="v")
                    nc.tensor.matmul(
                        out=v[:], lhsT=wst[:], rhs=u[:], start=True, stop=True
                    )
                    if lyr == 1:
                        # zd = v*dis^2 (+ b1*dis); tabT = lrelu(zd) = a1*dis
                        zd = wpool.tile([128, 128], F32, tag="zd", bufs=3, name="zd")
                        nc.vector.tensor_tensor(
                            out=zd[:],
                            in0=v[:],
                            in1=disq2T_s[:, cs],
                            op=mybir.AluOpType.mult,
                        )
                        if p.has_b1:
                            zd2 = wpool.tile(
                                [128, 128], F32, tag="zd2", bufs=3, name="zd2"
                            )
                            nc.vector.tensor_tensor(
                                out=zd2[:],
                                in0=zd[:],
                                in1=biasd1T_s[:, cs],
                                op=mybir.AluOpType.add,
                            )
                            zd = zd2
                        t3 = wpool.tile([128, 128], F32, tag="t3", bufs=3, name="t3")
                        nc.scalar.mul(out=t3[:], in_=zd[:], mul=NEG_SLOPE)
                        # tabT = lrelu(zd) = a1*dis: both the layer-2 table
                        # value AND the layer-2 self term -> write directly
                        # into the persistent selfnextT staging
                        nc.vector.tensor_tensor(
                            out=selfnextT[:, cs],
                            in0=zd[:],
                            in1=t3[:],
                            op=mybir.AluOpType.max,
                        )
                        # row-major staging for the AllGather input via PE
                        # transpose (XBAR DMA-transpose serializes against
                        # in-flight SWDGE gathers - measured ~2x layer-1 cost)
                        tp = pspool.tile(
                            [128, 128], BF16, space="PSUM", tag="tp", bufs=2,
                            name="tp",
                        )
                        nc.tensor.transpose(
                            out=tp[:], in_=selfnextT[:, cs], identity=ident_s[:]
                        )
                        nc.scalar.copy(out=a1rows[:, b, :], in_=tp[:])
                        # stream this block's AllGather input rows out now so
                        # the collective isn't gated on one big end-of-layer
                        # DMA (shrinks the pre-AG bubble)
                        nc.sync.dma_start(
                            out=agin_t[b * 128 : (b + 1) * 128, :],
                            in_=a1rows[:, b, :],
                        )
                    else:
                        t = wpool.tile([128, 128], F32, tag="t", bufs=3, name="t")
                        nc.vector.tensor_tensor(
                            out=t[:],
                            in0=v[:],
                            in1=disoT_s[:, cs],
                            op=mybir.AluOpType.mult,
                        )
                        nc.scalar.activation(
                            out=outst[:, b, :],
                            in_=t[:],
                            func=mybir.ActivationFunctionType.Identity,
                            bias=b2col_s[:, 0:1],
                        )

            emit_layer(1, [])
            nc.gpsimd.collective_compute(
                "AllGather",
                mybir.AluOpType.bypass,
                replica_groups=[list(range(NCORES))],
                ins=[agin_t[:, :].opt()],
                outs=[agout_t[:, :].opt()],
            )
            # pre-generate descriptors for the first L2 gather calls: emitted
            # AFTER the collective (the prep's deferred agout read must be
            # program-order-after the collective's write for the trigger to
            # inherit the dependency), but their Q7 generation runs during the
            # AllGather since prepare_only defers the data wait to the trigger.
            w2 = min(_PREPW1 + _PREPW2, len(p.calls))
            gts2 = [emit_gather(2, gi, prep=True) for gi in range(w2)]
            # engine-blocking fence: the trigger's inherited read-dep on agout
            # proved insufficient on HW - block the Pool engine behind the
            # collective via a real dependency chain: HWDGE reads agout into
            # SBUF (waits for the collective), then a Pool op consumes it.
            # (Must NOT be a SWDGE DMA: a non-prep DMA on a queue holding
            # untriggered preps would fire THEIR descriptors instead.)
            fsb = wpool.tile([1, 128], BF16, tag="agf", bufs=1, name="agf")
            nc.sync.dma_start(out=fsb[:, :], in_=agout_t[0:1, :])
            fsb2 = wpool.tile([1, 128], BF16, tag="agf2", bufs=1, name="agf2")
            nc.gpsimd.tensor_copy(out=fsb2[:, :], in_=fsb[:, :])
            for q in range(_NQUEUES):
                nq = sum(1 for gi in range(w2) if gi % _NQUEUES == q)
                if nq:
                    nc.gpsimd.trigger_dma(count=None, queue_num=q)
                    # Tile does not wire data-consumer waits to prepped
                    # gathers' DMA completion; gate the consumer engine (PE
                    # reads the gathered tiles as matmul weights) manually.
                    nc.tensor.wait_ge(qsems[q], 16 * nq)
            emit_layer(2, gts2)
            nc.sync.dma_start(out=out_d.ap(), in_=outst[:, :, :])

    nc.compile()
    return nc


_CACHE = {}


def _get_program(plan):
    nc = _CACHE.get(plan.key)
    if nc is None:
        nc = build_program(plan)
        _CACHE[plan.key] = nc
    return nc


def kernel(x, edge_index, batch, W1, b1, W2, b2):
    from concourse.bass_utils import run_bass_kernel_spmd

    x = np.asarray(x, np.float32)
    edge_index = np.asarray(edge_index)
    plan = make_plan(x.shape[0], edge_index)
    in_maps = make_in_maps(
        plan,
        x,
        np.asarray(W1, np.float32),
        np.asarray(b1, np.float32),
        np.asarray(W2, np.float32),
        np.asarray(b2, np.float32),
    )
    nc = _get_program(plan)
    res = run_bass_kernel_spmd(nc, in_maps, core_ids=list(range(NCORES)))
    big = np.concatenate(
        [res.results[k]["out"] for k in range(NCORES)], axis=1
    )  # [128, NPAD]
    out = big[:, plan.newid[: plan.N]].T
    return np.ascontiguousarray(out).astype(np.float32)
